# revision 7
# baseline (speedup 1.0000x reference)
"""Trainium2 Bass kernel for nn_LsqNonneg: batched NNLS via 200-iteration FISTA.

Math (matches reference.py exactly, in fp32):
    AtA = A.T @ A                       [32,32]
    L   = ||AtA||_2,  step = 1/L
    B   = step * A.T @ X                [32, N]
    W   = I - step*AtA                  [32,32]
    S_1 = relu(B);  S_0 = 0
    for k = 1..199:
        mu_k   = (t_{k-1}-1)/t_k        (t_0=1, t_k = 0.5(1+sqrt(1+4 t_{k-1}^2)))
        S_{k+1} = relu( (1+mu_k) W S_k  -  mu_k W S_{k-1}  +  B )   # momentum folded
    return S_200

Device layout (per core, NS=4096 columns):
    S stored as [128, NS/4]: partition group g (rows 32g..32g+31) holds columns
    [g*NS/4, (g+1)*NS/4); so one [128, 512] slice carries 4 independent column
    blocks stacked in partitions.  Weights are block-diagonal diag4(W) [128,128]
    so a single full-array matmul advances 4 column blocks at once with a full
    128-wide PSUM drain (one bank per 512-col slice).
    Per iteration per 512-slice: 3 accumulating matmuls into one PSUM bank:
       ident128 @ B   (start=True: writes B)
       diag4((1+mu)W) @ S_cur   (accumulate)
       diag4((-mu)W)  @ S_prev  (accumulate, stop)
    then relu(psum) -> S buffer (VectorE on slice 0, ScalarE on slice 1 so the
    two PSUM banks are read in parallel).  Per-iteration folded weights are
    streamed from DRAM (double-buffered), precomputed on host.
"""

import os
import sys

import numpy as np

for _p in ("/opt/trn_rl_repo", "/root/.axon_site/_ro/trn_rl_repo"):
    if os.path.isdir(_p) and _p not in sys.path:
        sys.path.append(_p)

from contextlib import ExitStack

import concourse.bass as bass
import concourse.bacc as bacc
import concourse.tile as tile
from concourse import mybir
from concourse.bass_utils import run_bass_kernel_spmd

M, K, N_FULL, N_CORES, ITERS = 512, 32, 32768, 8, 200

F32 = mybir.dt.float32
F32R = mybir.dt.float32r

# mm dtype for the PE: float32r runs at 1 cycle/row (vs 4 for float32) but with
# reduced precision on hardware; measured empirically via test.py.
MM_DTYPE = F32R

LAST_RESULTS = None  # BassKernelResults of the most recent run (for test.py)


def _mm(ap, dt_):
    return ap.bitcast(dt_) if dt_ is not F32 else ap


def build_program(ns: int, iters: int, mm_dtype=MM_DTYPE):
    """Build the SPMD Bass program for one core holding `ns` columns."""
    DT = mm_dtype
    q = ns // 4          # free extent of the packed [128, q] S layout
    nsl = q // 512       # number of 512-wide slices (PSUM banks per generation)
    assert ns % 2048 == 0 and nsl >= 1

    nc = bacc.Bacc("TRN2", target_bir_lowering=False)

    x_d = nc.dram_tensor("x", [M, ns], F32, kind="ExternalInput")
    apad_d = nc.dram_tensor("apad", [4, M, 128], F32, kind="ExternalInput")
    wd_d = nc.dram_tensor("wd", [max(iters - 1, 1), 2, 128, 128], F32,
                          kind="ExternalInput")
    id_d = nc.dram_tensor("ident", [128, 128], F32, kind="ExternalInput")
    out_d = nc.dram_tensor("s_out", [K, ns], F32, kind="ExternalOutput")

    with ExitStack() as ctx:
        tc = ctx.enter_context(tile.TileContext(nc))
        persist = ctx.enter_context(tc.tile_pool(name="persist", bufs=1))
        xpool = ctx.enter_context(tc.tile_pool(name="xstage", bufs=4))
        wpool = ctx.enter_context(tc.tile_pool(name="wstage", bufs=4))
        psum = ctx.enter_context(tc.tile_pool(name="psum", bufs=2, space="PSUM"))

        s_a = persist.tile([128, q], DT)   # S_odd  generations
        s_b = persist.tile([128, q], DT)   # S_even generations
        b_sb = persist.tile([128, q], DT)  # B in packed layout
        id_sb = persist.tile([128, 128], DT)
        

        nc.sync.dma_start(id_sb[:], id_d[:].bitcast(DT))
        for g in range(4):
            # apad[g] is [512,128]; chunk c -> [128,128] at free 128c... but we
            # need lhsT chunks [128m, 128] per (g, c): store as 4 tiles worth in
            # one [128, 512] region per g?  Simpler: one tile per (g,c) region:
            # ap_sb holds chunk layout [128, (g*? )] -- we instead DMA per (g,c).
            pass
        apc = persist.tile([128, 16 * 128], DT)  # (g,c) chunk at free 128*(4g+c)
        for g in range(4):
            for c in range(4):
                nc.sync.dma_start(
                    apc[:, 128 * (4 * g + c):128 * (4 * g + c + 1)],
                    apad_d[g, 128 * c:128 * (c + 1), :].bitcast(DT),
                )

        # ---- prologue: B = As.T @ X, packed layout, plus S_1 = relu(B) ----
        pb = psum.tile([128, q], F32)
        for c in range(4):
            xt = xpool.tile([128, ns], DT)
            nc.sync.dma_start(xt[:], x_d[128 * c:128 * (c + 1), :].bitcast(DT))
            for g in range(4):
                lhs = apc[:, 128 * (4 * g + c):128 * (4 * g + c + 1)]
                for s in range(nsl):
                    nc.tensor.matmul(
                        pb[:, 512 * s:512 * (s + 1)],
                        lhs,
                        xt[:, g * q + 512 * s: g * q + 512 * (s + 1)],
                        start=(c == 0 and g == 0),
                        stop=(c == 3 and g == 3),
                    )
        for s in range(nsl):
            sl = slice(512 * s, 512 * (s + 1))
            if s % 2 == 0:
                nc.vector.tensor_copy(b_sb[:, sl], pb[:, sl])
                nc.scalar.activation(s_a[:, sl], pb[:, sl],
                                     mybir.ActivationFunctionType.Relu)
            else:
                nc.scalar.copy(b_sb[:, sl], pb[:, sl])
                nc.vector.tensor_scalar_max(s_a[:, sl], pb[:, sl], 0.0)

        # ---- FISTA loop: k = 1..iters-1 computes S_{k+1} ----
        for k in range(1, iters):
            wt = wpool.tile([128, 256], DT)
            nc.sync.dma_start(wt[:].rearrange("p (w m) -> p w m", w=2),
                              wd_d[k - 1].rearrange("w p m -> p w m").bitcast(DT))
            cur, prev = (s_a, s_b) if k % 2 == 1 else (s_b, s_a)
            dest = prev
            pt = psum.tile([128, q], F32)
            for s in range(nsl):
                sl = slice(512 * s, 512 * (s + 1))
                nc.tensor.matmul(pt[:, sl], id_sb[:],
                                 b_sb[:, sl],
                                 start=True, stop=False)
                nc.tensor.matmul(pt[:, sl], wt[:, 0:128],
                                 cur[:, sl],
                                 start=False, stop=(k == 1))
                if k > 1:
                    nc.tensor.matmul(pt[:, sl], wt[:, 128:256],
                                     prev[:, sl],
                                     start=False, stop=True)
            for s in range(nsl):
                sl = slice(512 * s, 512 * (s + 1))
                if s % 2 == 0:
                    nc.vector.tensor_scalar_max(dest[:, sl], pt[:, sl], 0.0)
                else:
                    nc.scalar.activation(dest[:, sl], pt[:, sl],
                                         mybir.ActivationFunctionType.Relu)

        final = s_a if iters % 2 == 1 else s_b
        if iters == 1:
            final = s_a
        for g in range(4):
            for s in range(nsl):
                nc.sync.dma_start(
                    out_d[:, g * q + 512 * s: g * q + 512 * (s + 1)],
                    final[32 * g:32 * (g + 1), 512 * s:512 * (s + 1)].bitcast(F32),
                )

    nc.finalize()
    return nc


def host_prep(A: np.ndarray, iters: int):
    """Replicate the reference's fp32 scalar math and build device weights."""
    A = np.asarray(A, dtype=np.float32)
    AtA = (A.T @ A).astype(np.float32)
    L = np.linalg.svd(AtA, compute_uv=False)[0].astype(np.float32)
    step = (np.float32(1.0) / L).astype(np.float32)
    W = (np.eye(K, dtype=np.float32) - step * AtA).astype(np.float32)
    As = (step * A).astype(np.float32)

    # t/mu sequence in fp32 exactly like the reference scan
    t = np.float32(1.0)
    mus = []
    for _ in range(1, iters):
        t_new = (np.float32(0.5) * (np.float32(1.0) +
                 np.sqrt(np.float32(1.0) + np.float32(4.0) * t * t))).astype(np.float32)
        mus.append(((t - np.float32(1.0)) / t_new).astype(np.float32))
        t = t_new

    # folded per-iteration block-diagonal weights (lhsT = diag4(scaled W).T)
    Wt = W.T.astype(np.float64)
    wd = np.zeros((max(iters - 1, 1), 2, 128, 128), dtype=np.float32)
    for i, mu in enumerate(mus):
        wc = ((1.0 + np.float64(mu)) * Wt).astype(np.float32)
        wp = ((-np.float64(mu)) * Wt).astype(np.float32)
        for g in range(4):
            wd[i, 0, 32 * g:32 * (g + 1), 32 * g:32 * (g + 1)] = wc
            wd[i, 1, 32 * g:32 * (g + 1), 32 * g:32 * (g + 1)] = wp

    apad = np.zeros((4, M, 128), dtype=np.float32)
    for g in range(4):
        apad[g, :, 32 * g:32 * (g + 1)] = As
    ident = np.eye(128, dtype=np.float32)
    return wd, apad, ident


_PROGRAM_CACHE = {}


def _get_program(ns, iters):
    key = (ns, iters, str(MM_DTYPE))
    if key not in _PROGRAM_CACHE:
        _PROGRAM_CACHE[key] = build_program(ns, iters)
    return _PROGRAM_CACHE[key]


def kernel(X: np.ndarray, A: np.ndarray) -> np.ndarray:
    global LAST_RESULTS
    X = np.ascontiguousarray(np.asarray(X, dtype=np.float32))
    A = np.ascontiguousarray(np.asarray(A, dtype=np.float32))
    assert X.shape == (M, N_FULL) and A.shape == (M, K)

    ns = N_FULL // N_CORES
    wd, apad, ident = host_prep(A, ITERS)
    nc = _get_program(ns, ITERS)

    in_maps = []
    for c in range(N_CORES):
        in_maps.append({
            "x": np.ascontiguousarray(X[:, c * ns:(c + 1) * ns]),
            "apad": apad,
            "wd": wd,
            "ident": ident,
        })

    res = run_bass_kernel_spmd(nc, in_maps, core_ids=list(range(N_CORES)))
    LAST_RESULTS = res
    S = np.concatenate([res.results[c]["s_out"] for c in range(N_CORES)], axis=1)
    return S.astype(np.float32)
